# revision 32
# baseline (speedup 1.0000x reference)
"""Trainium2 Bass kernel for causal multi-head attention (fused QKV + attention + out-proj).

Problem: nn_MultiHeadAttention_68771016343935
  B=2, S=2048, D=1024, H=16 heads, dk=64, causal mask, f32 I/O.

Sharding (8 cores): core c handles batch b=c//4, head group hg=c%4 (4 heads),
processed as 2 head-pairs of 128 partition-dims. w_qkv column-sharded by head,
w_o row-sharded; partial outputs summed on host (the "all-reduce").

Device algorithm per core (scores kept TRANSPOSED so softmax reductions come
out of the tensor engine for free):
  1. QKV projection: Q^T,K^T per pair [128, 2048] from xT (host-pretransposed
     query) with column-sliced w_qkv; V^T transposed back to natural V via PE
     and augmented with a ones-column (V_aug [k,65]).
  2. Per q-block (512) x k-tile (128), causal-trimmed:
     S^T[k,q] = K^T.T @ Q^T (both heads row-tiled on the PE),
     P^T = exp(SCALE * S^T) (ACT, PSUM->SBUF, fp32r round),
     diagonal k-tiles multiplied by a triu mask,
     ctx^T[65,512] += V_aug.T @ P^T  (row 64 accumulates Z = sum_k P).
  3. Normalize: 1/Z broadcast along partitions via a K=1 matmul with a ones
     column, ctx^T scaled on DVE.
  4. Out-proj: O[s,e] = ctx^T.T @ w_o rows, accumulated over the 2 pairs.

Matmuls run in float32r (single-pass PE mode, ~11-bit mantissa, 4x faster
than native fp32); set KERNEL_MM_F32=1 for the full-fp32 build.
"""

import os
import numpy as np

B, S, D = 2, 2048, 1024
H, DK = 16, 64
SCALE = 1.0 / 8.0
NEG = -1000000000.0
NQB, QB = 4, 512      # q blocks
NKT, KT = 16, 128     # k tiles
NDT = 8               # d tiles of 128 in the contraction over D

_compiled = {}


def _numpy_ref(query, mask, w_qkv, w_o):
    Bq, Sq, Dq = query.shape
    Hh = Dq // DK if Dq % DK == 0 else H
    qkv = query.reshape(-1, Dq).astype(np.float32) @ w_qkv.astype(np.float32)
    qkv = qkv.reshape(Bq, Sq, -1)
    q, k, v = np.split(qkv, 3, axis=-1)

    def heads(x):
        return x.reshape(Bq, Sq, Hh, -1).transpose(0, 2, 1, 3)

    Q, K, V = heads(q), heads(k), heads(v)
    scale = np.float32(1.0 / np.sqrt(Q.shape[-1]))
    scores = np.einsum("bhqd,bhkd->bhqk", Q, K).astype(np.float32) * scale
    scores = np.where(np.asarray(mask) == 0, np.float32(NEG), scores)
    scores = scores - scores.max(axis=-1, keepdims=True)
    e = np.exp(scores, dtype=np.float32)
    attn = e / e.sum(axis=-1, keepdims=True)
    ctx = np.einsum("bhqk,bhkd->bhqd", attn, V).astype(np.float32)
    ctx = ctx.transpose(0, 2, 1, 3).reshape(Bq, Sq, Dq)
    return ctx @ w_o.astype(np.float32)


def _build(use_f32, dbg=False):
    import concourse.bacc as bacc
    import concourse.tile as tile
    from concourse import mybir
    from concourse.masks import make_identity

    dt = mybir.dt
    F32 = dt.float32
    MDT = dt.float32 if use_f32 else dt.float32r
    Exp = mybir.ActivationFunctionType.Exp

    nc = bacc.Bacc(None, target_bir_lowering=False, debug=False)
    xT_d = nc.dram_tensor("xT", [D, S], MDT, kind="ExternalInput")
    wq_d = nc.dram_tensor("wqkv", [D, 768], MDT, kind="ExternalInput")
    wo_d = nc.dram_tensor("wo", [256, D], MDT, kind="ExternalInput")
    out_d = nc.dram_tensor("opart", [S, D], F32, kind="ExternalOutput")
    if dbg:
        dbg_qt = nc.dram_tensor("dbg_qt", [128, S], MDT, kind="ExternalOutput")
        dbg_kt = nc.dram_tensor("dbg_kt", [128, S], MDT, kind="ExternalOutput")
        dbg_va = nc.dram_tensor("dbg_va", [128, 2 * NKT, 65], MDT, kind="ExternalOutput")
        dbg_ct = nc.dram_tensor("dbg_ct", [128, S], MDT, kind="ExternalOutput")
        dbg_nm = nc.dram_tensor("dbg_nm", [128, 128], F32, kind="ExternalOutput")
        dbg_pt0 = nc.dram_tensor("dbg_pt0", [128, 1024], MDT, kind="ExternalOutput")
        dbg_pt1 = nc.dram_tensor("dbg_pt1", [128, 1024], MDT, kind="ExternalOutput")
        dbg_cu = nc.dram_tensor("dbg_cu", [130, S], F32, kind="ExternalOutput")
        dbg_rz = nc.dram_tensor("dbg_rz", [1, 512], F32, kind="ExternalOutput")
        dbg_bc = nc.dram_tensor("dbg_bc", [64, 512], F32, kind="ExternalOutput")
        dbg_o1 = nc.dram_tensor("dbg_o1", [1, 64], F32, kind="ExternalOutput")

    with tile.TileContext(nc) as tc, \
         tc.tile_pool(name="persist", bufs=1) as persist, \
         tc.tile_pool(name="vtp", bufs=1) as vtp, \
         tc.tile_pool(name="ptp", bufs=(2 if dbg else 3)) as ptp, \
         tc.tile_pool(name="rzp", bufs=2) as rzp, \
         tc.tile_pool(name="osbp", bufs=2) as osbp:

        # ---- constants ----
        ident = persist.tile([128, 128], F32, tag="ident")
        make_identity(nc, ident[:])
        # additive causal mask for diagonal k-tiles in S^T orientation:
        # negmask[k, q] = 0 where k <= q else NEG
        negmask = persist.tile([128, 128], F32, tag="negmask")
        nc.gpsimd.memset(negmask[:], 0.0)
        nc.gpsimd.affine_select(
            out=negmask[:],
            in_=negmask[:],
            compare_op=mybir.AluOpType.is_ge,
            fill=NEG,
            base=0,
            # keep in_ (0) where (q - k) >= 0, i.e. k <= q; else NEG
            pattern=[[1, 128]],
            channel_multiplier=-1,
        )
        # 0/1 upper-triangular (incl diagonal) mask for P^T diag tiles
        triu_f = persist.tile([128, 128], F32, tag="triu_f")
        nc.gpsimd.memset(triu_f[:], 0.0)
        nc.gpsimd.affine_select(
            out=triu_f[:],
            in_=triu_f[:],
            compare_op=mybir.AluOpType.is_gt,
            fill=1.0,
            base=0,
            # (k - q) > 0 keeps in_ (0); else fill 1.0  =>  1 where k <= q
            pattern=[[-1, 128]],
            channel_multiplier=1,
        )
        triu_r = persist.tile([128, 128], MDT, tag="triu_r")
        nc.vector.tensor_copy(triu_r[:], triu_f[:])
        ones3 = persist.tile([128, 2 * NKT, 1], F32, tag="ones3")
        nc.gpsimd.memset(ones3[:], 1.0)
        ones_f = persist.tile([1, 64], F32, tag="ones_f")
        nc.gpsimd.memset(ones_f[:], 1.0)
        ones64 = persist.tile([1, 64], MDT, tag="ones64")
        nc.vector.tensor_copy(ones64[:], ones_f[:])

        # ---- input DMAs: wqkv coalesced to one transfer, xT tiles
        # alternated across both HWDGE rings (sync + scalar) so descriptor
        # issue doesn't serialize; wo goes last (only needed by out-proj) ----
        wq, xt = [], []
        for t in range(NDT):
            wtile = persist.tile([128, 768], MDT, tag=f"wq{t}", name=f"wq{t}")
            nc.scalar.dma_start(wtile[:], wq_d[128 * t : 128 * (t + 1), :])
            wq.append(wtile)
            xtile = persist.tile([128, S], MDT, tag=f"xt{t}", name=f"xt{t}")
            nc.sync.dma_start(xtile[:], xT_d[128 * t : 128 * (t + 1), :])
            xt.append(xtile)
        wo_big = persist.tile([128, 2, D], MDT, tag="wo_big")
        nc.scalar.dma_start(wo_big[:], wo_d[:].rearrange("(t p) c -> p t c", p=128))
        wo = [wo_big[:, p, :] for p in range(2)]

        # ---- phase 1: QKV projection + V transpose/augment ----
        QT2, KT2, VAUG, CTXT = [], [], [], []
        for p in range(2):
            qt = persist.tile([128, S], MDT, tag=f"qt{p}")
            kt = persist.tile([128, S], MDT, tag=f"kt{p}")
            QT2.append(qt)
            KT2.append(kt)
            ctxt = persist.tile([128, S], MDT, tag=f"ctxt{p}")
            CTXT.append(ctxt)

        vts = []
        for p in range(2):
            vts.append(vtp.tile([128, S], F32, tag="vt", bufs=1, name=f"vt{p}"))

        # QKV as 4 waves of 6 concurrent accumulation groups, d-tile loop
        # outermost, so the PE streams matmuls as each xT tile's DMA lands
        # instead of stalling on one 8-deep accumulation at a time.
        groups = []  # (p, slot, sc)
        for p in range(2):
            for slot in range(3):
                for sc in range(4):
                    groups.append((p, slot, sc))

        with tc.tile_pool(name="ps_qkv", bufs=1, space="PSUM") as ps1:
            for w in range(4):
                wave = groups[6 * w : 6 * w + 6]
                pmms = [
                    ps1.tile([128, 512], F32, tag="qkv", bufs=6, name=f"pmm{w}{g}")
                    for g in range(6)
                ]
                for t in range(NDT):
                    for g, (p, slot, sc) in enumerate(wave):
                        wcol = 128 * (3 * p + slot)
                        nc.tensor.matmul(
                            pmms[g][:],
                            wq[t][:, wcol : wcol + 128],
                            xt[t][:, 512 * sc : 512 * (sc + 1)],
                            start=(t == 0),
                            stop=(t == NDT - 1),
                        )
                for g, (p, slot, sc) in enumerate(wave):
                    dst = (QT2[p], KT2[p], vts[p])[slot]
                    nc.scalar.copy(dst[:, 512 * sc : 512 * (sc + 1)], pmms[g][:])

                if w == 1 or w == 3:
                    # pair w//2's V is complete: transpose to natural V + ones
                    p = w // 2
                    vaug = persist.tile([128, 2 * NKT, 65], MDT, tag=f"vaug{p}", name=f"vaug{p}")
                    VAUG.append(vaug)
                    nc.vector.tensor_copy(vaug[:, :, 64:65], ones3[:])
                    for t in range(NKT):
                        ptr = ps1.tile([128, 128], F32, tag="tr", bufs=2, name=f"tr{p}{t}")
                        nc.tensor.transpose(
                            ptr[:], vts[p][:, 128 * t : 128 * (t + 1)], ident[:]
                        )
                        for h in range(2):
                            nc.vector.tensor_copy(
                                vaug[:, 2 * t + h, 0:64], ptr[:, 64 * h : 64 * h + 64]
                            )

        # ---- phase 2: attention + out-projection per q-block ----
        # The two head-pairs' k-loops are interleaved event-by-event so the
        # PE always has a second independent stream while one pair waits on
        # its exp; keeps the tensor engine dense (HAM stays un-throttled).
        with tc.tile_pool(name="ps2", bufs=1, space="PSUM") as ps2:
            for j in range(NQB):
                ctx_ps = {}
                for p in range(2):
                    for h in range(2):
                        ctx_ps[(p, h)] = ps2.tile(
                            [65, 512], F32, tag="ctx", bufs=4, name=f"ctx{j}{p}{h}"
                        )
                ntile = 4 * j + 4
                for t in range(ntile):
                    r = t - 4 * j
                    q0 = max(QB * j, KT * t)
                    off = q0 - QB * j
                    N = QB - off
                    for p in range(2):
                        s2 = ps2.tile(
                            [128, 1024], F32, tag="s2", bufs=2, name=f"s2{j}{t}{p}"
                        )
                        for h in range(2):
                            nc.tensor.matmul(
                                s2[:, 512 * h : 512 * h + N],
                                KT2[p][64 * h : 64 * h + 64, KT * t : KT * (t + 1)],
                                QT2[p][64 * h : 64 * h + 64, q0 : q0 + N],
                                start=True,
                                stop=True,
                            )
                        pT = ptp.tile([128, 2 * N], MDT, tag="pT", name=f"pT{j}{t}{p}")
                        if N == 512:
                            nc.scalar.activation(pT[:], s2[:], Exp, scale=SCALE)
                        else:
                            for h in range(2):
                                nc.scalar.activation(
                                    pT[:, h * N : (h + 1) * N],
                                    s2[:, 512 * h : 512 * h + N],
                                    Exp,
                                    scale=SCALE,
                                )
                        if r >= 0:
                            # zero the masked (k > q) half of the diagonal
                            # sub-block; on GpSimd to keep the DVE FIFO clear
                            for h in range(2):
                                nc.gpsimd.tensor_mul(
                                    pT[:, h * N : h * N + 128],
                                    pT[:, h * N : h * N + 128],
                                    triu_r[:],
                                )
                        if dbg and p == 0 and j == 0 and t == 0:
                            nc.sync.dma_start(dbg_pt0[:], pT[:])
                        if dbg and p == 0 and j == 1 and t == 0:
                            nc.sync.dma_start(dbg_pt1[:], pT[:])
                        for h in range(2):
                            nc.tensor.matmul(
                                ctx_ps[(p, h)][:, off:QB],
                                VAUG[p][:, 2 * t + h, :],
                                pT[:, h * N : (h + 1) * N],
                                start=(t == 0),
                                stop=(t == ntile - 1),
                            )

                # normalize all four (p, h) chains
                for p in range(2):
                    for h in range(2):
                        cu = rzp.tile(
                            [65, 512], F32, tag="cu", bufs=4, name=f"cu{j}{p}{h}"
                        )
                        nc.scalar.copy(cu[:], ctx_ps[(p, h)][:])
                        rz_f = rzp.tile([1, 512], F32, tag="rzvec", bufs=2, name=f"rzf{j}{p}{h}")
                        rz_s = rzp.tile([1, 512], F32, tag="rzvec", bufs=2, name=f"rzs{j}{p}{h}")
                        nc.vector.reciprocal_approx_accurate(rz_f[:], cu[64:65, :], rz_s[:])
                        rz = rzp.tile([1, 512], MDT, tag="rz", bufs=2, name=f"rz{j}{p}{h}")
                        nc.gpsimd.tensor_copy(rz[:], rz_f[:])
                        bc = ps2.tile([64, 512], F32, tag="s2", bufs=2, name=f"bc{j}{p}{h}")
                        nc.tensor.matmul(bc[:], ones64[:], rz[:], start=True, stop=True)
                        if dbg and j == 0 and p == 0 and h == 0:
                            nc.sync.dma_start(dbg_rz[:], rz_f[:])
                            nc.sync.dma_start(dbg_o1[:], ones_f[:])
                        nc.vector.tensor_mul(
                            CTXT[p][64 * h : 64 * h + 64, QB * j : QB * (j + 1)],
                            cu[0:64, :],
                            bc[:],
                        )
                if dbg and p == 1:
                    for h in range(2):
                        pass

                if dbg and j == NQB - 1:
                    nc.sync.dma_start(dbg_qt[:], QT2[0][:])
                    nc.sync.dma_start(dbg_kt[:], KT2[0][:])
                    nc.sync.dma_start(dbg_va[:], VAUG[0][:])
                    nc.sync.dma_start(dbg_ct[:], CTXT[0][:])
                    nc.sync.dma_start(dbg_nm[:], negmask[:])

                # out-projection for this q-block
                for st in range(4):
                    s0 = QB * j + 128 * st
                    osb = osbp.tile([128, D], F32, tag="osb", name=f"osb{j}{st}")
                    opsl = [
                        ps2.tile([128, 512], F32, tag="s2", bufs=2, name=f"ops{j}{st}{eh}")
                        for eh in range(2)
                    ]
                    for p in range(2):
                        for eh in range(2):
                            nc.tensor.matmul(
                                opsl[eh][:],
                                CTXT[p][:, s0 : s0 + 128],
                                wo[p][:, 512 * eh : 512 * (eh + 1)],
                                start=(p == 0),
                                stop=(p == 1),
                            )
                    nc.vector.tensor_copy(osb[:, 0:512], opsl[0][:])
                    nc.vector.tensor_copy(osb[:, 512:1024], opsl[1][:])
                    nc.sync.dma_start(out_d[s0 : s0 + 128, :], osb[:])

    nc.compile()
    return nc


def _get_nc():
    use_f32 = os.environ.get("KERNEL_MM_F32", "0") == "1"
    key = ("f32" if use_f32 else "f32r",)
    if key not in _compiled:
        _compiled[key] = _build(use_f32)
    return _compiled[key]


def kernel(**inputs):
    query = np.asarray(inputs["query"], dtype=np.float32)
    mask = np.asarray(inputs["mask"])
    w_qkv = np.asarray(inputs["w_qkv"], dtype=np.float32)
    w_o = np.asarray(inputs["w_o"], dtype=np.float32)

    ok = (
        query.shape == (B, S, D)
        and mask.shape == (1, 1, S, S)
        and w_qkv.shape == (D, 3 * D)
        and w_o.shape == (D, D)
        and np.array_equal(
            np.asarray(mask[0, 0]) != 0, np.tril(np.ones((S, S), dtype=bool))
        )
    )
    if not ok:
        return _numpy_ref(query, mask, w_qkv, w_o).astype(np.float32)

    from concourse.bass_utils import run_bass_kernel_spmd

    nc = _get_nc()
    in_maps = []
    xTs = [np.ascontiguousarray(query[b].T) for b in range(B)]
    for c in range(8):
        b, hg = c // 4, c % 4
        h0 = 4 * hg
        cols = []
        for p in range(2):
            hA = h0 + 2 * p
            for slot in range(3):
                base = slot * D
                cols.append(w_qkv[:, base + hA * DK : base + (hA + 2) * DK])
        wq_c = np.ascontiguousarray(np.concatenate(cols, axis=1))
        wo_c = np.ascontiguousarray(w_o[h0 * DK : (h0 + 4) * DK, :])
        in_maps.append({"xT": xTs[b], "wqkv": wq_c, "wo": wo_c})

    try:
        res = run_bass_kernel_spmd(nc, in_maps, list(range(8)))
    except Exception:
        # transient axon/device failure: reset the accelerator and retry once
        import time as _time

        try:
            import ctypes

            _lib = ctypes.CDLL("/opt/axon/libaxon_pjrt.so")
            _lib.axon_reset.restype = ctypes.c_int64
            _lib.axon_reset()
        except Exception:
            pass
        _time.sleep(10)
        res = run_bass_kernel_spmd(nc, in_maps, list(range(8)))
    out = np.zeros((B, S, D), dtype=np.float32)
    for c in range(8):
        out[c // 4] += res.results[c]["opart"]
    return out


# revision 33
# speedup vs baseline: 1.0149x; 1.0149x over previous
"""Trainium2 Bass kernel for causal multi-head attention (fused QKV + attention + out-proj).

Problem: nn_MultiHeadAttention_68771016343935
  B=2, S=2048, D=1024, H=16 heads, dk=64, causal mask, f32 I/O.

Sharding (8 cores): core c handles batch b=c//4, head group hg=c%4 (4 heads),
processed as 2 head-pairs of 128 partition-dims. w_qkv column-sharded by head,
w_o row-sharded; partial outputs summed on host (the "all-reduce").

Device algorithm per core (scores kept TRANSPOSED so softmax reductions come
out of the tensor engine for free):
  1. QKV projection: Q^T,K^T per pair [128, 2048] from xT (host-pretransposed
     query) with column-sliced w_qkv; V^T transposed back to natural V via PE
     and augmented with a ones-column (V_aug [k,65]).
  2. Per q-block (512) x k-tile (128), causal-trimmed:
     S^T[k,q] = K^T.T @ Q^T (both heads row-tiled on the PE),
     P^T = exp(SCALE * S^T) (ACT, PSUM->SBUF, fp32r round),
     diagonal k-tiles multiplied by a triu mask,
     ctx^T[65,512] += V_aug.T @ P^T  (row 64 accumulates Z = sum_k P).
  3. Normalize: 1/Z broadcast along partitions via a K=1 matmul with a ones
     column, ctx^T scaled on DVE.
  4. Out-proj: O[s,e] = ctx^T.T @ w_o rows, accumulated over the 2 pairs.

Matmuls run in float32r (single-pass PE mode, ~11-bit mantissa, 4x faster
than native fp32); set KERNEL_MM_F32=1 for the full-fp32 build.
"""

import os
import numpy as np

B, S, D = 2, 2048, 1024
H, DK = 16, 64
SCALE = 1.0 / 8.0
NEG = -1000000000.0
NQB, QB = 4, 512      # q blocks
NKT, KT = 16, 128     # k tiles
NDT = 8               # d tiles of 128 in the contraction over D

_compiled = {}


def _numpy_ref(query, mask, w_qkv, w_o):
    Bq, Sq, Dq = query.shape
    Hh = Dq // DK if Dq % DK == 0 else H
    qkv = query.reshape(-1, Dq).astype(np.float32) @ w_qkv.astype(np.float32)
    qkv = qkv.reshape(Bq, Sq, -1)
    q, k, v = np.split(qkv, 3, axis=-1)

    def heads(x):
        return x.reshape(Bq, Sq, Hh, -1).transpose(0, 2, 1, 3)

    Q, K, V = heads(q), heads(k), heads(v)
    scale = np.float32(1.0 / np.sqrt(Q.shape[-1]))
    scores = np.einsum("bhqd,bhkd->bhqk", Q, K).astype(np.float32) * scale
    scores = np.where(np.asarray(mask) == 0, np.float32(NEG), scores)
    scores = scores - scores.max(axis=-1, keepdims=True)
    e = np.exp(scores, dtype=np.float32)
    attn = e / e.sum(axis=-1, keepdims=True)
    ctx = np.einsum("bhqk,bhkd->bhqd", attn, V).astype(np.float32)
    ctx = ctx.transpose(0, 2, 1, 3).reshape(Bq, Sq, Dq)
    return ctx @ w_o.astype(np.float32)


def _build(use_f32, dbg=False):
    import concourse.bacc as bacc
    import concourse.tile as tile
    from concourse import mybir
    from concourse.masks import make_identity

    dt = mybir.dt
    F32 = dt.float32
    MDT = dt.float32 if use_f32 else dt.float32r
    Exp = mybir.ActivationFunctionType.Exp

    nc = bacc.Bacc(None, target_bir_lowering=False, debug=False)
    xT_d = nc.dram_tensor("xT", [D, S], MDT, kind="ExternalInput")
    wq_d = nc.dram_tensor("wqkv", [D, 768], MDT, kind="ExternalInput")
    wo_d = nc.dram_tensor("wo", [256, D], MDT, kind="ExternalInput")
    out_d = nc.dram_tensor("opart", [S, D], F32, kind="ExternalOutput")
    if dbg:
        dbg_qt = nc.dram_tensor("dbg_qt", [128, S], MDT, kind="ExternalOutput")
        dbg_kt = nc.dram_tensor("dbg_kt", [128, S], MDT, kind="ExternalOutput")
        dbg_va = nc.dram_tensor("dbg_va", [128, 2 * NKT, 65], MDT, kind="ExternalOutput")
        dbg_ct = nc.dram_tensor("dbg_ct", [128, S], MDT, kind="ExternalOutput")
        dbg_nm = nc.dram_tensor("dbg_nm", [128, 128], F32, kind="ExternalOutput")
        dbg_pt0 = nc.dram_tensor("dbg_pt0", [128, 1024], MDT, kind="ExternalOutput")
        dbg_pt1 = nc.dram_tensor("dbg_pt1", [128, 1024], MDT, kind="ExternalOutput")
        dbg_cu = nc.dram_tensor("dbg_cu", [130, S], F32, kind="ExternalOutput")
        dbg_rz = nc.dram_tensor("dbg_rz", [1, 512], F32, kind="ExternalOutput")
        dbg_bc = nc.dram_tensor("dbg_bc", [64, 512], F32, kind="ExternalOutput")
        dbg_o1 = nc.dram_tensor("dbg_o1", [1, 64], F32, kind="ExternalOutput")

    with tile.TileContext(nc) as tc, \
         tc.tile_pool(name="persist", bufs=1) as persist, \
         tc.tile_pool(name="vtp", bufs=1) as vtp, \
         tc.tile_pool(name="ptp", bufs=(2 if dbg else 3)) as ptp, \
         tc.tile_pool(name="rzp", bufs=2) as rzp, \
         tc.tile_pool(name="osbp", bufs=2) as osbp:

        # ---- constants ----
        ident = persist.tile([128, 128], F32, tag="ident")
        make_identity(nc, ident[:])
        # additive causal mask for diagonal k-tiles in S^T orientation:
        # negmask[k, q] = 0 where k <= q else NEG
        negmask = persist.tile([128, 128], F32, tag="negmask")
        nc.gpsimd.memset(negmask[:], 0.0)
        nc.gpsimd.affine_select(
            out=negmask[:],
            in_=negmask[:],
            compare_op=mybir.AluOpType.is_ge,
            fill=NEG,
            base=0,
            # keep in_ (0) where (q - k) >= 0, i.e. k <= q; else NEG
            pattern=[[1, 128]],
            channel_multiplier=-1,
        )
        # 0/1 upper-triangular (incl diagonal) mask for P^T diag tiles
        triu_f = persist.tile([128, 128], F32, tag="triu_f")
        nc.gpsimd.memset(triu_f[:], 0.0)
        nc.gpsimd.affine_select(
            out=triu_f[:],
            in_=triu_f[:],
            compare_op=mybir.AluOpType.is_gt,
            fill=1.0,
            base=0,
            # (k - q) > 0 keeps in_ (0); else fill 1.0  =>  1 where k <= q
            pattern=[[-1, 128]],
            channel_multiplier=1,
        )
        triu_r = persist.tile([128, 128], MDT, tag="triu_r")
        nc.vector.tensor_copy(triu_r[:], triu_f[:])
        ones3 = persist.tile([128, 2 * NKT, 1], F32, tag="ones3")
        nc.gpsimd.memset(ones3[:], 1.0)
        ones_f = persist.tile([1, 64], F32, tag="ones_f")
        nc.gpsimd.memset(ones_f[:], 1.0)
        ones64 = persist.tile([1, 64], MDT, tag="ones64")
        nc.vector.tensor_copy(ones64[:], ones_f[:])

        # ---- input DMAs: wqkv coalesced to one transfer, xT tiles
        # alternated across both HWDGE rings (sync + scalar) so descriptor
        # issue doesn't serialize; wo goes last (only needed by out-proj) ----
        wq, xt = [], []
        for t in range(NDT):
            wtile = persist.tile([128, 768], MDT, tag=f"wq{t}", name=f"wq{t}")
            nc.scalar.dma_start(wtile[:], wq_d[128 * t : 128 * (t + 1), :])
            wq.append(wtile)
            xtile = persist.tile([128, S], MDT, tag=f"xt{t}", name=f"xt{t}")
            nc.sync.dma_start(xtile[:], xT_d[128 * t : 128 * (t + 1), :])
            xt.append(xtile)
        wo_big = persist.tile([128, 2, D], MDT, tag="wo_big")
        nc.scalar.dma_start(wo_big[:], wo_d[:].rearrange("(t p) c -> p t c", p=128))
        wo = [wo_big[:, p, :] for p in range(2)]

        # ---- phase 1: QKV projection + V transpose/augment ----
        QT2, KT2, VAUG, CTXT = [], [], [], []
        for p in range(2):
            qt = persist.tile([128, S], MDT, tag=f"qt{p}")
            kt = persist.tile([128, S], MDT, tag=f"kt{p}")
            QT2.append(qt)
            KT2.append(kt)
            ctxt = persist.tile([128, S], MDT, tag=f"ctxt{p}")
            CTXT.append(ctxt)

        vts = []
        for p in range(2):
            vts.append(vtp.tile([128, S], F32, tag="vt", bufs=1, name=f"vt{p}"))

        # QKV as 4 waves of 6 concurrent accumulation groups, d-tile loop
        # outermost, so the PE streams matmuls as each xT tile's DMA lands
        # instead of stalling on one 8-deep accumulation at a time.
        groups = []  # (p, slot, sc)
        for p in range(2):
            for slot in range(3):
                for sc in range(4):
                    groups.append((p, slot, sc))

        with tc.tile_pool(name="ps_qkv", bufs=1, space="PSUM") as ps1:
            for w in range(4):
                wave = groups[6 * w : 6 * w + 6]
                pmms = [
                    ps1.tile([128, 512], F32, tag="qkv", bufs=6, name=f"pmm{w}{g}")
                    for g in range(6)
                ]
                for t in range(NDT):
                    for g, (p, slot, sc) in enumerate(wave):
                        wcol = 128 * (3 * p + slot)
                        nc.tensor.matmul(
                            pmms[g][:],
                            wq[t][:, wcol : wcol + 128],
                            xt[t][:, 512 * sc : 512 * (sc + 1)],
                            start=(t == 0),
                            stop=(t == NDT - 1),
                        )
                for g, (p, slot, sc) in enumerate(wave):
                    dst = (QT2[p], KT2[p], vts[p])[slot]
                    nc.scalar.copy(dst[:, 512 * sc : 512 * (sc + 1)], pmms[g][:])

                if w == 1 or w == 3:
                    # pair w//2's V is complete: transpose to natural V + ones
                    p = w // 2
                    vaug = persist.tile([128, 2 * NKT, 65], MDT, tag=f"vaug{p}", name=f"vaug{p}")
                    VAUG.append(vaug)
                    nc.vector.tensor_copy(vaug[:, :, 64:65], ones3[:])
                    for t in range(NKT):
                        ptr = ps1.tile([128, 128], F32, tag="tr", bufs=2, name=f"tr{p}{t}")
                        nc.tensor.transpose(
                            ptr[:], vts[p][:, 128 * t : 128 * (t + 1)], ident[:]
                        )
                        for h in range(2):
                            nc.vector.tensor_copy(
                                vaug[:, 2 * t + h, 0:64], ptr[:, 64 * h : 64 * h + 64]
                            )

        # ---- phase 2: attention + out-projection per q-block ----
        # The two head-pairs' k-loops are interleaved event-by-event so the
        # PE always has a second independent stream while one pair waits on
        # its exp; keeps the tensor engine dense (HAM stays un-throttled).
        with tc.tile_pool(name="ps2", bufs=1, space="PSUM") as ps2:
            for j in range(NQB):
                ctx_ps = {}
                for p in range(2):
                    for h in range(2):
                        ctx_ps[(p, h)] = ps2.tile(
                            [65, 512], F32, tag="ctx", bufs=4, name=f"ctx{j}{p}{h}"
                        )
                ntile = 4 * j + 4
                for t in range(ntile):
                    r = t - 4 * j
                    q0 = max(QB * j, KT * t)
                    off = q0 - QB * j
                    N = QB - off
                    for p in range(2):
                        s2 = ps2.tile(
                            [128, 1024], F32, tag="s2", bufs=2, name=f"s2{j}{t}{p}"
                        )
                        for h in range(2):
                            nc.tensor.matmul(
                                s2[:, 512 * h : 512 * h + N],
                                KT2[p][64 * h : 64 * h + 64, KT * t : KT * (t + 1)],
                                QT2[p][64 * h : 64 * h + 64, q0 : q0 + N],
                                start=True,
                                stop=True,
                            )
                        pT = ptp.tile([128, 2 * N], MDT, tag="pT", name=f"pT{j}{t}{p}")
                        if N == 512:
                            nc.scalar.activation(pT[:], s2[:], Exp, scale=SCALE)
                        else:
                            for h in range(2):
                                nc.scalar.activation(
                                    pT[:, h * N : (h + 1) * N],
                                    s2[:, 512 * h : 512 * h + N],
                                    Exp,
                                    scale=SCALE,
                                )
                        if r >= 0:
                            # zero the masked (k > q) half of the diagonal
                            # sub-block; on GpSimd to keep the DVE FIFO clear
                            for h in range(2):
                                nc.gpsimd.tensor_mul(
                                    pT[:, h * N : h * N + 128],
                                    pT[:, h * N : h * N + 128],
                                    triu_r[:],
                                )
                        if dbg and p == 0 and j == 0 and t == 0:
                            nc.sync.dma_start(dbg_pt0[:], pT[:])
                        if dbg and p == 0 and j == 1 and t == 0:
                            nc.sync.dma_start(dbg_pt1[:], pT[:])
                        for h in range(2):
                            nc.tensor.matmul(
                                ctx_ps[(p, h)][:, off:QB],
                                VAUG[p][:, 2 * t + h, :],
                                pT[:, h * N : (h + 1) * N],
                                start=(t == 0),
                                stop=(t == ntile - 1),
                            )

                # normalize all four (p, h) chains
                for p in range(2):
                    for h in range(2):
                        cu = rzp.tile(
                            [65, 512], F32, tag="cu", bufs=4, name=f"cu{j}{p}{h}"
                        )
                        nc.scalar.copy(cu[:], ctx_ps[(p, h)][:])
                        rz_f = rzp.tile([1, 512], F32, tag="rzvec", bufs=2, name=f"rzf{j}{p}{h}")
                        rz_s = rzp.tile([1, 512], F32, tag="rzvec", bufs=2, name=f"rzs{j}{p}{h}")
                        nc.vector.reciprocal_approx_accurate(rz_f[:], cu[64:65, :], rz_s[:])
                        rz = rzp.tile([1, 512], MDT, tag="rz", bufs=2, name=f"rz{j}{p}{h}")
                        nc.vector.tensor_copy(rz[:], rz_f[:])
                        bc = ps2.tile([64, 512], F32, tag="s2", bufs=2, name=f"bc{j}{p}{h}")
                        nc.tensor.matmul(bc[:], ones64[:], rz[:], start=True, stop=True)
                        if dbg and j == 0 and p == 0 and h == 0:
                            nc.sync.dma_start(dbg_rz[:], rz_f[:])
                            nc.sync.dma_start(dbg_o1[:], ones_f[:])
                        nc.vector.tensor_mul(
                            CTXT[p][64 * h : 64 * h + 64, QB * j : QB * (j + 1)],
                            cu[0:64, :],
                            bc[:],
                        )
                if dbg and p == 1:
                    for h in range(2):
                        pass

                if dbg and j == NQB - 1:
                    nc.sync.dma_start(dbg_qt[:], QT2[0][:])
                    nc.sync.dma_start(dbg_kt[:], KT2[0][:])
                    nc.sync.dma_start(dbg_va[:], VAUG[0][:])
                    nc.sync.dma_start(dbg_ct[:], CTXT[0][:])
                    nc.sync.dma_start(dbg_nm[:], negmask[:])

                # out-projection for this q-block
                for st in range(4):
                    s0 = QB * j + 128 * st
                    osb = osbp.tile([128, D], F32, tag="osb", name=f"osb{j}{st}")
                    opsl = [
                        ps2.tile([128, 512], F32, tag="s2", bufs=2, name=f"ops{j}{st}{eh}")
                        for eh in range(2)
                    ]
                    for p in range(2):
                        for eh in range(2):
                            nc.tensor.matmul(
                                opsl[eh][:],
                                CTXT[p][:, s0 : s0 + 128],
                                wo[p][:, 512 * eh : 512 * (eh + 1)],
                                start=(p == 0),
                                stop=(p == 1),
                            )
                    nc.vector.tensor_copy(osb[:, 0:512], opsl[0][:])
                    nc.vector.tensor_copy(osb[:, 512:1024], opsl[1][:])
                    nc.sync.dma_start(out_d[s0 : s0 + 128, :], osb[:])

    nc.compile()
    return nc


def _get_nc():
    use_f32 = os.environ.get("KERNEL_MM_F32", "0") == "1"
    key = ("f32" if use_f32 else "f32r",)
    if key not in _compiled:
        _compiled[key] = _build(use_f32)
    return _compiled[key]


def kernel(**inputs):
    query = np.asarray(inputs["query"], dtype=np.float32)
    mask = np.asarray(inputs["mask"])
    w_qkv = np.asarray(inputs["w_qkv"], dtype=np.float32)
    w_o = np.asarray(inputs["w_o"], dtype=np.float32)

    ok = (
        query.shape == (B, S, D)
        and mask.shape == (1, 1, S, S)
        and w_qkv.shape == (D, 3 * D)
        and w_o.shape == (D, D)
        and np.array_equal(
            np.asarray(mask[0, 0]) != 0, np.tril(np.ones((S, S), dtype=bool))
        )
    )
    if not ok:
        return _numpy_ref(query, mask, w_qkv, w_o).astype(np.float32)

    from concourse.bass_utils import run_bass_kernel_spmd

    nc = _get_nc()
    in_maps = []
    xTs = [np.ascontiguousarray(query[b].T) for b in range(B)]
    for c in range(8):
        b, hg = c // 4, c % 4
        h0 = 4 * hg
        cols = []
        for p in range(2):
            hA = h0 + 2 * p
            for slot in range(3):
                base = slot * D
                cols.append(w_qkv[:, base + hA * DK : base + (hA + 2) * DK])
        wq_c = np.ascontiguousarray(np.concatenate(cols, axis=1))
        wo_c = np.ascontiguousarray(w_o[h0 * DK : (h0 + 4) * DK, :])
        in_maps.append({"xT": xTs[b], "wqkv": wq_c, "wo": wo_c})

    try:
        res = run_bass_kernel_spmd(nc, in_maps, list(range(8)))
    except Exception:
        # transient axon/device failure: reset the accelerator and retry once
        import time as _time

        try:
            import ctypes

            _lib = ctypes.CDLL("/opt/axon/libaxon_pjrt.so")
            _lib.axon_reset.restype = ctypes.c_int64
            _lib.axon_reset()
        except Exception:
            pass
        _time.sleep(10)
        res = run_bass_kernel_spmd(nc, in_maps, list(range(8)))
    out = np.zeros((B, S, D), dtype=np.float32)
    for c in range(8):
        out[c // 4] += res.results[c]["opart"]
    return out


# revision 34
# speedup vs baseline: 1.2932x; 1.2743x over previous
"""Trainium2 Bass kernel for causal multi-head attention (fused QKV + attention + out-proj).

Problem: nn_MultiHeadAttention_68771016343935
  B=2, S=2048, D=1024, H=16 heads, dk=64, causal mask, f32 I/O.

Sharding (8 cores): core c handles batch b=c//4, head group hg=c%4 (4 heads),
processed as 2 head-pairs of 128 partition-dims. w_qkv column-sharded by head,
w_o row-sharded; partial outputs summed on host (the "all-reduce").

Device algorithm per core (scores kept TRANSPOSED so softmax reductions come
out of the tensor engine for free):
  1. QKV projection: Q^T,K^T per pair [128, 2048] from xT (host-pretransposed
     query) with column-sliced w_qkv; V^T transposed back to natural V via PE
     and augmented with a ones-column (V_aug [k,65]).
  2. Per q-block (512) x k-tile (128), causal-trimmed:
     S^T[k,q] = K^T.T @ Q^T (both heads row-tiled on the PE),
     P^T = exp(SCALE * S^T) (ACT, PSUM->SBUF, fp32r round),
     diagonal k-tiles multiplied by a triu mask,
     ctx^T[65,512] += V_aug.T @ P^T  (row 64 accumulates Z = sum_k P).
  3. Normalize: 1/Z broadcast along partitions via a K=1 matmul with a ones
     column, ctx^T scaled on DVE.
  4. Out-proj: O[s,e] = ctx^T.T @ w_o rows, accumulated over the 2 pairs.

Matmuls run in float32r (single-pass PE mode, ~11-bit mantissa, 4x faster
than native fp32); set KERNEL_MM_F32=1 for the full-fp32 build.
"""

import os
import numpy as np

B, S, D = 2, 2048, 1024
H, DK = 16, 64
SCALE = 1.0 / 8.0
NEG = -1000000000.0
NQB, QB = 4, 512      # q blocks
NKT, KT = 16, 128     # k tiles
NDT = 8               # d tiles of 128 in the contraction over D

_compiled = {}


def _numpy_ref(query, mask, w_qkv, w_o):
    Bq, Sq, Dq = query.shape
    Hh = Dq // DK if Dq % DK == 0 else H
    qkv = query.reshape(-1, Dq).astype(np.float32) @ w_qkv.astype(np.float32)
    qkv = qkv.reshape(Bq, Sq, -1)
    q, k, v = np.split(qkv, 3, axis=-1)

    def heads(x):
        return x.reshape(Bq, Sq, Hh, -1).transpose(0, 2, 1, 3)

    Q, K, V = heads(q), heads(k), heads(v)
    scale = np.float32(1.0 / np.sqrt(Q.shape[-1]))
    scores = np.einsum("bhqd,bhkd->bhqk", Q, K).astype(np.float32) * scale
    scores = np.where(np.asarray(mask) == 0, np.float32(NEG), scores)
    scores = scores - scores.max(axis=-1, keepdims=True)
    e = np.exp(scores, dtype=np.float32)
    attn = e / e.sum(axis=-1, keepdims=True)
    ctx = np.einsum("bhqk,bhkd->bhqd", attn, V).astype(np.float32)
    ctx = ctx.transpose(0, 2, 1, 3).reshape(Bq, Sq, Dq)
    return ctx @ w_o.astype(np.float32)


def _build(use_f32, dbg=False):
    import concourse.bacc as bacc
    import concourse.tile as tile
    from concourse import mybir
    from concourse.masks import make_identity

    dt = mybir.dt
    F32 = dt.float32
    MDT = dt.float32 if use_f32 else dt.float32r
    Exp = mybir.ActivationFunctionType.Exp
    DEPTH = 6  # ctx-matmul deferral (in (t,p) events) behind scores/exp

    nc = bacc.Bacc(None, target_bir_lowering=False, debug=False)
    xT_d = nc.dram_tensor("xT", [D, S], MDT, kind="ExternalInput")
    wq_d = nc.dram_tensor("wqkv", [D, 768], MDT, kind="ExternalInput")
    wo_d = nc.dram_tensor("wo", [256, D], MDT, kind="ExternalInput")
    out_d = nc.dram_tensor("opart", [S, D], F32, kind="ExternalOutput")

    with tile.TileContext(nc) as tc, \
         tc.tile_pool(name="persist", bufs=1) as persist:

        # ---- constants ----
        ident = persist.tile([128, 128], F32, tag="ident")
        make_identity(nc, ident[:])
        # 0/1 upper-triangular (incl diagonal): 1 where k <= q
        triu_f = persist.tile([128, 128], F32, tag="triu_f")
        nc.gpsimd.memset(triu_f[:], 0.0)
        nc.gpsimd.affine_select(
            out=triu_f[:],
            in_=triu_f[:],
            compare_op=mybir.AluOpType.is_gt,
            fill=1.0,
            base=0,
            pattern=[[-1, 128]],
            channel_multiplier=1,
        )
        triu_r = persist.tile([128, 128], MDT, tag="triu_r")
        nc.vector.tensor_copy(triu_r[:], triu_f[:])
        ones3 = persist.tile([128, 2 * NKT, 1], F32, tag="ones3")
        nc.gpsimd.memset(ones3[:], 1.0)
        ones_f = persist.tile([1, 64], F32, tag="ones_f")
        nc.gpsimd.memset(ones_f[:], 1.0)
        ones64 = persist.tile([1, 64], MDT, tag="ones64")
        nc.vector.tensor_copy(ones64[:], ones_f[:])

        QT2, KT2, VAUG, CTXT = [], [], [], []
        for p in range(2):
            QT2.append(persist.tile([128, S], MDT, tag=f"qt{p}", name=f"qt{p}"))
            KT2.append(persist.tile([128, S], MDT, tag=f"kt{p}", name=f"kt{p}"))
            CTXT.append(persist.tile([128, S], MDT, tag=f"ctxt{p}", name=f"ctxt{p}"))
        wo_big = persist.tile([128, 2, D], MDT, tag="wo_big")

        # ---- phase 1: input DMAs + QKV projection + V transpose ----
        with tc.tile_pool(name="p1sb", bufs=1) as p1sb, \
             tc.tile_pool(name="ps1", bufs=1, space="PSUM") as ps1:
            wq, xt = [], []
            for t in range(NDT):
                wtile = p1sb.tile([128, 768], MDT, tag=f"wq{t}", name=f"wq{t}")
                nc.scalar.dma_start(wtile[:], wq_d[128 * t : 128 * (t + 1), :])
                wq.append(wtile)
                xtile = p1sb.tile([128, S], MDT, tag=f"xt{t}", name=f"xt{t}")
                nc.sync.dma_start(xtile[:], xT_d[128 * t : 128 * (t + 1), :])
                xt.append(xtile)
            nc.scalar.dma_start(
                wo_big[:], wo_d[:].rearrange("(t p) c -> p t c", p=128)
            )

            vts = [
                p1sb.tile([128, S], F32, tag="vt", bufs=1, name=f"vt{p}")
                for p in range(2)
            ]
            groups = [
                (p, slot, sc)
                for p in range(2)
                for slot in range(3)
                for sc in range(4)
            ]
            for w in range(4):
                wave = groups[6 * w : 6 * w + 6]
                pmms = [
                    ps1.tile([128, 512], F32, tag="qkv", bufs=6, name=f"pmm{w}{g}")
                    for g in range(6)
                ]
                for t in range(NDT):
                    for g, (p, slot, sc) in enumerate(wave):
                        wcol = 128 * (3 * p + slot)
                        nc.tensor.matmul(
                            pmms[g][:],
                            wq[t][:, wcol : wcol + 128],
                            xt[t][:, 512 * sc : 512 * (sc + 1)],
                            start=(t == 0),
                            stop=(t == NDT - 1),
                        )
                for g, (p, slot, sc) in enumerate(wave):
                    dst = (QT2[p], KT2[p], vts[p])[slot]
                    nc.scalar.copy(dst[:, 512 * sc : 512 * (sc + 1)], pmms[g][:])

                if w in (1, 3):
                    p = w // 2
                    vaug = persist.tile(
                        [128, 2 * NKT, 65], MDT, tag=f"vaug{p}", name=f"vaug{p}"
                    )
                    VAUG.append(vaug)
                    nc.vector.tensor_copy(vaug[:, :, 64:65], ones3[:])
                    for t in range(NKT):
                        ptr = ps1.tile(
                            [128, 128], F32, tag="tr", bufs=2, name=f"tr{p}{t}"
                        )
                        nc.tensor.transpose(
                            ptr[:], vts[p][:, 128 * t : 128 * (t + 1)], ident[:]
                        )
                        for h in range(2):
                            nc.vector.tensor_copy(
                                vaug[:, 2 * t + h, 0:64],
                                ptr[:, 64 * h : 64 * h + 64],
                            )

        # ---- phase 2: attention + out-proj, software-pipelined ----
        wo = [wo_big[:, p, :] for p in range(2)]
        with tc.tile_pool(name="ptp", bufs=1) as ptp, \
             tc.tile_pool(name="rzp", bufs=1) as rzp, \
             tc.tile_pool(name="osbp", bufs=4) as osbp, \
             tc.tile_pool(name="ps2", bufs=1, space="PSUM") as ps2:

            def emit_outproj_st(j, st):
                s0 = QB * j + 128 * st
                osb = osbp.tile([128, D], F32, tag="osb", name=f"osb{j}{st}")
                opsl = [
                    ps2.tile(
                        [128, 512], F32, tag="s2", bufs=2, name=f"ops{j}{st}{eh}"
                    )
                    for eh in range(2)
                ]
                for p in range(2):
                    for eh in range(2):
                        nc.tensor.matmul(
                            opsl[eh][:],
                            CTXT[p][:, s0 : s0 + 128],
                            wo[p][:, 512 * eh : 512 * (eh + 1)],
                            start=(p == 0),
                            stop=(p == 1),
                        )
                nc.vector.tensor_copy(osb[:, 0:512], opsl[0][:])
                nc.vector.tensor_copy(osb[:, 512:1024], opsl[1][:])
                nc.sync.dma_start(out_d[s0 : s0 + 128, :], osb[:])

            def emit_norm_tail(j, p, h, chain):
                # bc matmul + ctxT scale; chain = (cu, rz)
                cu, rz = chain
                bc = ps2.tile(
                    [64, 512], F32, tag="s2", bufs=2, name=f"bc{j}{p}{h}"
                )
                nc.tensor.matmul(bc[:], ones64[:], rz[:], start=True, stop=True)
                nc.vector.tensor_mul(
                    CTXT[p][64 * h : 64 * h + 64, QB * j : QB * (j + 1)],
                    cu[0:64, :],
                    bc[:],
                )

            norm_chains = {}   # (j, p, h) -> (cu, rz)
            deferred = []      # list of emit thunks carried into next block

            for j in range(NQB):
                ctx_ps = {
                    (p, h): ps2.tile(
                        [65, 512], F32, tag="ctx", bufs=4, name=f"ctx{j}{p}{h}"
                    )
                    for p in range(2)
                    for h in range(2)
                }
                ntile = 4 * j + 4
                events = [(t, p) for t in range(ntile) for p in range(2)]
                pend = []
                defq = list(deferred)
                deferred = []

                def flush_ctx():
                    t, p, pT, off, N = pend.pop(0)
                    for h in range(2):
                        nc.tensor.matmul(
                            ctx_ps[(p, h)][:, off:QB],
                            VAUG[p][:, 2 * t + h, :],
                            pT[:, h * N : (h + 1) * N],
                            start=(t == 0),
                            stop=(t == ntile - 1),
                        )

                for idx, (t, p) in enumerate(events):
                    r = t - 4 * j
                    q0 = max(QB * j, KT * t)
                    off = q0 - QB * j
                    N = QB - off
                    s2 = ps2.tile(
                        [128, 1024], F32, tag="s2", bufs=2, name=f"s2{j}{t}{p}"
                    )
                    for h in range(2):
                        nc.tensor.matmul(
                            s2[:, 512 * h : 512 * h + N],
                            KT2[p][64 * h : 64 * h + 64, KT * t : KT * (t + 1)],
                            QT2[p][64 * h : 64 * h + 64, q0 : q0 + N],
                            start=True,
                            stop=True,
                        )
                    pT = ptp.tile(
                        [128, 2 * N], MDT, tag="pT", bufs=DEPTH + 3,
                        name=f"pT{j}{t}{p}",
                    )
                    if N == 512:
                        nc.scalar.activation(pT[:], s2[:], Exp, scale=SCALE)
                    else:
                        for h in range(2):
                            nc.scalar.activation(
                                pT[:, h * N : (h + 1) * N],
                                s2[:, 512 * h : 512 * h + N],
                                Exp,
                                scale=SCALE,
                            )
                    if r >= 0:
                        for h in range(2):
                            nc.gpsimd.tensor_mul(
                                pT[:, h * N : h * N + 128],
                                pT[:, h * N : h * N + 128],
                                triu_r[:],
                            )
                    pend.append((t, p, pT, off, N))
                    if len(pend) > DEPTH:
                        flush_ctx()
                    # weave previous block's bc/mult/out-proj into this stream
                    if defq and idx >= 2 and idx % 2 == 0:
                        defq.pop(0)()
                while pend:
                    flush_ctx()
                for fn in defq:
                    fn()

                # normalize heads: free ctx psum, compute 1/Z; bc deferred
                for p in range(2):
                    for h in range(2):
                        cu = rzp.tile(
                            [65, 512], F32, tag="cu", bufs=5, name=f"cu{j}{p}{h}"
                        )
                        nc.scalar.copy(cu[:], ctx_ps[(p, h)][:])
                        rz_f = rzp.tile(
                            [1, 512], F32, tag="rzf", bufs=5, name=f"rzf{j}{p}{h}"
                        )
                        rz_s = rzp.tile(
                            [1, 512], F32, tag="rzs", bufs=2, name=f"rzs{j}{p}{h}"
                        )
                        nc.vector.reciprocal_approx_accurate(
                            rz_f[:], cu[64:65, :], rz_s[:]
                        )
                        rz = rzp.tile(
                            [1, 512], MDT, tag="rz", bufs=5, name=f"rz{j}{p}{h}"
                        )
                        nc.vector.tensor_copy(rz[:], rz_f[:])
                        norm_chains[(j, p, h)] = (cu, rz)

                def mk_tail(j):
                    thunks = []
                    for p in range(2):
                        for h in range(2):
                            thunks.append(
                                lambda j=j, p=p, h=h: emit_norm_tail(
                                    j, p, h, norm_chains[(j, p, h)]
                                )
                            )
                    for st in range(4):
                        thunks.append(lambda j=j, st=st: emit_outproj_st(j, st))
                    return thunks

                if j < NQB - 1:
                    deferred = mk_tail(j)
                else:
                    for fn in mk_tail(j):
                        fn()

    nc.compile()
    return nc


def _get_nc():
    use_f32 = os.environ.get("KERNEL_MM_F32", "0") == "1"
    key = ("f32" if use_f32 else "f32r",)
    if key not in _compiled:
        _compiled[key] = _build(use_f32)
    return _compiled[key]


def kernel(**inputs):
    query = np.asarray(inputs["query"], dtype=np.float32)
    mask = np.asarray(inputs["mask"])
    w_qkv = np.asarray(inputs["w_qkv"], dtype=np.float32)
    w_o = np.asarray(inputs["w_o"], dtype=np.float32)

    ok = (
        query.shape == (B, S, D)
        and mask.shape == (1, 1, S, S)
        and w_qkv.shape == (D, 3 * D)
        and w_o.shape == (D, D)
        and np.array_equal(
            np.asarray(mask[0, 0]) != 0, np.tril(np.ones((S, S), dtype=bool))
        )
    )
    if not ok:
        return _numpy_ref(query, mask, w_qkv, w_o).astype(np.float32)

    from concourse.bass_utils import run_bass_kernel_spmd

    nc = _get_nc()
    in_maps = []
    xTs = [np.ascontiguousarray(query[b].T) for b in range(B)]
    for c in range(8):
        b, hg = c // 4, c % 4
        h0 = 4 * hg
        cols = []
        for p in range(2):
            hA = h0 + 2 * p
            for slot in range(3):
                base = slot * D
                cols.append(w_qkv[:, base + hA * DK : base + (hA + 2) * DK])
        wq_c = np.ascontiguousarray(np.concatenate(cols, axis=1))
        wo_c = np.ascontiguousarray(w_o[h0 * DK : (h0 + 4) * DK, :])
        in_maps.append({"xT": xTs[b], "wqkv": wq_c, "wo": wo_c})

    try:
        res = run_bass_kernel_spmd(nc, in_maps, list(range(8)))
    except Exception:
        # transient axon/device failure: reset the accelerator and retry once
        import time as _time

        try:
            import ctypes

            _lib = ctypes.CDLL("/opt/axon/libaxon_pjrt.so")
            _lib.axon_reset.restype = ctypes.c_int64
            _lib.axon_reset()
        except Exception:
            pass
        _time.sleep(10)
        res = run_bass_kernel_spmd(nc, in_maps, list(range(8)))
    out = np.zeros((B, S, D), dtype=np.float32)
    for c in range(8):
        out[c // 4] += res.results[c]["opart"]
    return out
